# revision 1
# baseline (speedup 1.0000x reference)
"""Multi-head causal attention (SEQ=4096, D=1024, H=16, DK=64) on 8 TRN2
NeuronCores, tensor-parallel over heads (2 heads/core). Self-contained.

Per-core pipeline:
  1. Projections: Qh^T/Kh^T/Vh^T = W.T @ X^T (X^T passed pre-transposed from
     host, 1/sqrt(dk) folded into W_Q host-side). bf16 copies kept for stats.
  2. Stats pass: S = Qh^T.T @ Kh^T in bf16, causal mask added via
     identity-matmul of a -1e9 mask tile, row-max reduced (bf16 error is
     harmless: softmax is shift-invariant, m only needs |m - max| << 80).
  3. S^T pass in exact fp32: S^T[kc,q] = [Kh;1].T @ [Qh;-m] (the max
     subtraction rides the contraction as a 65th row), mask via identity
     matmul, then ACT exp -> P^T. AV in f32r: O^T = [Vh|1].T-style
     ones-augmented Vh gives l = sum(exp) as row 64 of the PSUM accumulator.
  4. R = rank-2 broadcast of 1/l over head halves (PE), C^T scaled (DVE),
     Y_partial = C^T.T @ W_O_rows in f32r, DMA out. Host sums 8 partials.
"""

import os
import sys

sys.path.insert(0, "/opt/trn_rl_repo")

import numpy as np
import ml_dtypes

import concourse.bass as bass
import concourse.mybir as mybir
import concourse.tile as tile
from concourse.bass_utils import run_bass_kernel_spmd
from concourse.masks import make_identity

P = 128
S = 4096
D = 1024
DK = 64
NH = 2  # heads per core
NCORES = 8
NEG = -1.0e9
F32 = mybir.dt.float32
F32R = mybir.dt.float32r
BF16 = mybir.dt.bfloat16
EXP = mybir.ActivationFunctionType.Exp

_ctr = [0]


def _split_waits(nc, max_waits=1):
    """walrus rejects >1 sem-wait per instruction; move extras onto
    preceding same-engine NOPs (engine streams are program-ordered)."""
    for f in nc.m.functions:
        for bb in f.blocks:
            insts = bb.instructions
            new = []
            changed = False
            for inst in insts:
                si = inst.sync_info
                if si is not None and si.on_wait and len(si.on_wait) > max_waits:
                    waits = list(si.on_wait)
                    extra, keep = waits[:-max_waits], waits[-max_waits:]
                    for i in range(0, len(extra), max_waits):
                        _ctr[0] += 1
                        new.append(
                            mybir.InstNoOp(
                                name=f"waitsplit-{_ctr[0]}",
                                engine=inst.engine,
                                ins=[],
                                outs=[],
                                sync_info=mybir.SyncInfo(
                                    on_wait=extra[i : i + max_waits], on_update=[]
                                ),
                            )
                        )
                    inst.sync_info = mybir.SyncInfo(
                        on_wait=keep, on_update=list(si.on_update)
                    )
                    changed = True
                new.append(inst)
            if changed:
                bb.instructions = new


def build(nc: bass.Bass, causal: bool = True):
    stages = int(os.environ.get("ATTN_STAGES", "6"))
    repeat = int(os.environ.get("ATTN_REPEAT", "1"))
    r_st = bool(int(os.environ.get("ATTN_R_ST", "0")))    # S^T pass in f32r
    r_proj = bool(int(os.environ.get("ATTN_R_PROJ", "1")))  # projections in f32r
    # S^T matmul passes: 3 = exact hi/lo (hi.hi + hi.lo + lo.hi), 1 = hi.hi
    # only. Softmax renormalization cancels the shared per-row error of the
    # top terms; measured 9.0e-3 rel err on HW vs the 2e-2 gate.
    st_passes = int(os.environ.get("ATTN_ST_PASSES", "1"))
    use_lo = st_passes >= 2
    comb_y = bool(int(os.environ.get("ATTN_COMB_Y", "1")))  # combine heads on device
    # pad stats/S^T stationaries with zero rows to contract over all 128
    # partitions: same matmul cost (cols x 1cyc), but keeps the PE array
    # fully fed so the HAM clock gate holds K=8/8 (2.4 GHz) instead of
    # dropping to 1.2 GHz.
    pad128 = bool(int(os.environ.get("ATTN_PAD128", "1"))) and st_passes == 1
    # fuse W_O over both heads: pre-scale ct by 1/l (PE rank-2 R + DVE mul),
    # then one 128-deep matmul per (qc, eb) instead of two 64-deep + ACT
    # scale + DVE add.
    wo_fuse = bool(int(os.environ.get("ATTN_WO_FUSE", "1")))
    F_IN = F32R if r_proj else F32
    F_QK = F32R if r_st else F32
    NB = S // 512  # 8   512-wide blocks
    QB = S // P  # 32  128-wide q blocks
    DC = D // P  # 8   128-deep contraction chunks

    qT = nc.dram_tensor("qT", [D, S], F_IN, kind="ExternalInput")
    kT = nc.dram_tensor("kT", [D, S], F_IN, kind="ExternalInput")
    vT = nc.dram_tensor("vT", [D, S], F32R, kind="ExternalInput")
    wq = nc.dram_tensor("wq", [D, NH * DK], F_IN, kind="ExternalInput")
    wk = nc.dram_tensor("wk", [D, NH * DK], F_IN, kind="ExternalInput")
    wv = nc.dram_tensor("wv", [D, NH * DK], F32R, kind="ExternalInput")
    wo = nc.dram_tensor("wo", [NH * DK, D], F32R, kind="ExternalInput")
    maskf = nc.dram_tensor("maskf", [P, P], BF16, kind="ExternalInput")
    maskb = nc.dram_tensor("maskb", [P, P], BF16, kind="ExternalInput")
    onesr = nc.dram_tensor("onesr", [1, S], F32R, kind="ExternalInput")
    zerosr = nc.dram_tensor("zerosr", [1, S], F32R, kind="ExternalInput")
    sel2d = nc.dram_tensor("sel2d", [33, P], F32R, kind="ExternalInput")
    zeros64 = nc.dram_tensor("zeros64", [DK, S], F32R, kind="ExternalInput")
    comb_pre = bool(int(os.environ.get("ATTN_COMB_Y", "1")))
    y0 = nc.dram_tensor("y0", [S, D], F32, kind="ExternalOutput")
    y1 = None if comb_pre else nc.dram_tensor("y1", [S, D], F32, kind="ExternalOutput")

    with tile.TileContext(nc) as tc:
        import contextlib

        ctx = contextlib.ExitStack()
        with ctx:
            const = ctx.enter_context(tc.tile_pool(name="const", bufs=1))
            big = ctx.enter_context(tc.tile_pool(name="big", bufs=1))
            stream = ctx.enter_context(tc.tile_pool(name="stream", bufs=int(os.environ.get("ATTN_BSTREAM", "8"))))
            ptp = ctx.enter_context(tc.tile_pool(name="ptp", bufs=int(os.environ.get("ATTN_BPT", "3"))))
            ypool = ctx.enter_context(tc.tile_pool(name="ypool", bufs=int(os.environ.get("ATTN_BY", "2"))))
            smalls = ctx.enter_context(tc.tile_pool(name="smalls", bufs=2))
            bproj = int(os.environ.get("ATTN_BPROJ", "2"))
            bstat = int(os.environ.get("ATTN_BSTAT", "2"))
            bst = int(os.environ.get("ATTN_BST", "2"))
            bmisc = int(os.environ.get("ATTN_BMISC", "2"))
            ps_proj = ctx.enter_context(
                tc.tile_pool(name="ps_proj", bufs=bproj, space="PSUM")
            )
            ps_stat = ctx.enter_context(
                tc.tile_pool(name="ps_stat", bufs=bstat, space="PSUM")
            )
            ps_st = ctx.enter_context(tc.tile_pool(name="ps_st", bufs=bst, space="PSUM"))
            ps_misc = ctx.enter_context(
                tc.tile_pool(name="ps_misc", bufs=bmisc, space="PSUM")
            )

            # ---- constants ----
            ident = const.tile([P, P], F32)
            make_identity(nc, ident[:])
            ident_b = const.tile([P, P], BF16)
            nc.vector.tensor_copy(ident_b[:], ident[:])
            ident_r = const.tile([P, P], F32R)
            nc.vector.tensor_copy(ident_r[:], ident[:])

            wq_sb = const.tile([P, DC, P], F_IN, tag="wq")
            wk_sb = const.tile([P, DC, P], F_IN, tag="wk")
            wv_sb = const.tile([P, DC, P], F32R, tag="wv")
            nc.sync.dma_start(wq_sb[:], wq.rearrange("(o p) m -> p o m", p=P))
            nc.sync.dma_start(wk_sb[:], wk.rearrange("(o p) m -> p o m", p=P))
            nc.sync.dma_start(wv_sb[:], wv.rearrange("(o p) m -> p o m", p=P))
            wo_sb = const.tile([P, D], F32R, tag="wo")
            nc.sync.dma_start(wo_sb[:], wo[:])

            mf_sb = const.tile([P, P], BF16, tag="mf")
            mb_sb = const.tile([P, P], BF16, tag="mb")
            nc.sync.dma_start(mf_sb[:], maskf[:])
            nc.sync.dma_start(mb_sb[:], maskb[:])

            # ---- persistent activations ----
            # exact QK via f32r hi/lo pairs: x = hi + lo to ~26 bits; the
            # 3-product matmul (hi.hi + hi.lo + lo.hi) is f32-exact (probed
            # 1.16e-7 on HW) at 3 cyc/row vs fp32's 4.
            qhT_hi = [big.tile([P, S], F32R, tag=f"qhTh{h}", name=f"qhTh{h}") for h in range(NH)]
            khT_hi = [big.tile([P, S], F32R, tag=f"khTh{h}", name=f"khTh{h}") for h in range(NH)]
            if use_lo:
                qhT_lo = [big.tile([P, S], F32R, tag=f"qhTl{h}", name=f"qhTl{h}") for h in range(NH)]
                khT_lo = [big.tile([P, S], F32R, tag=f"khTl{h}", name=f"khTl{h}") for h in range(NH)]
            else:
                qhT_lo = khT_lo = None
            vh = [big.tile([P, QB, DK + 1], F32R, tag=f"vh{h}", name=f"vh{h}") for h in range(NH)]
            ct = big.tile([P, S], F32R, tag="ct")
            mcol = [big.tile([P, QB], F32, tag=f"mcol{h}", name=f"mcol{h}") for h in range(NH)]
            mcol_hi = [big.tile([P, QB], F32R, tag=f"mcolh{h}", name=f"mcolh{h}") for h in range(NH)]
            mcol_lo = [big.tile([P, QB], F32R, tag=f"mcoll{h}", name=f"mcoll{h}") for h in range(NH)]
            # l rows live in dead partitions of qhT_hi[h] (row 96)
            lcol = [big.tile([P, QB], F32R, tag=f"lcol{h}", name=f"lcol{h}") for h in range(NH)]
            rcol = [big.tile([P, QB], F32, tag=f"rcol{h}", name=f"rcol{h}") for h in range(NH)]

            ones_qb = const.tile([P, QB], F32, tag="ones_qb")
            nc.any.memset(ones_qb[:], 1.0)
            # l scratch rows (row h) — kept OUT of qhT_hi so its padded rows
            # stay zero.
            lscr = const.tile([P, 512], F32R, tag="lscr")
            if wo_fuse:
                # head-half selector for the rank-2 R matmul: row 0 ->
                # output partitions 0..63, row 32 -> 64..127; rows 1..31 zero
                # (1/l rows live at partitions 0/32, legal DVE write bases).
                sel2 = const.tile([P, P], F32R, tag="sel2")
                nc.sync.dma_start(sel2[0:33, :], sel2d[:])
                # 1/l rows (row 32*h, cols nb*512..): written by st3_emit
                # straight off the AV accumulator's l row. Rows 1..31 ride
                # into the R matmul as moving junk x zero stationary — zero
                # them so no NaN poisons 0*x.
                lrT = big.tile([P, S], F32R, tag="lrT")
                nc.scalar.dma_start(lrT[0:33, :], zeros64[0:33, :])
            for h in range(NH):
                if pad128:
                    # zero pad rows once, on the ACT-side DMA queue so the
                    # 4.5MB doesn't delay the SP-queue proj input stream.
                    # (memset output trips the f32r rounding check, so DMA
                    # host zeros instead.)
                    nc.scalar.dma_start(qhT_hi[h][DK:P, :], zeros64[:])
                    nc.scalar.dma_start(khT_hi[h][DK:P, :], zeros64[:])
                nc.sync.dma_start(khT_hi[h][DK : DK + 1, :], onesr[:])  # ones row
                if use_lo:
                    nc.sync.dma_start(khT_lo[h][DK : DK + 1, :], zerosr[:])  # zero row
                nc.vector.tensor_copy(vh[h][:, :, DK], ones_qb[:])  # ones col


            for _rep in range(repeat):
                # ---- stage 1: Q/K projections (V is emitted per-round below).
                # nb-major order (Q-nb, K-nb, then head-0 stats for qb-block
                # nb): the proj phase is DMA-bound (~90us of streaming vs
                # ~40us of 1-pass matmul), so feeding the PE the head-0 stats
                # matmuls — whose khT deps are exactly blocks <= nb — absorbs
                # the idle. ----
                def proj_emit(t_idx, nb):
                    xdram, w_sb = [(qT, wq_sb), (kT, wk_sb)][t_idx]
                    ps = ps_proj.tile([P, 512], F32, tag="proj", name="ps")
                    for dc in range(DC):
                        xt = stream.tile([P, 512], F_IN, tag="xin", name="xt")
                        nc.sync.dma_start(
                            xt[:],
                            xdram[dc * P : (dc + 1) * P, nb * 512 : (nb + 1) * 512],
                        )
                        nc.tensor.matmul(
                            ps[:],
                            w_sb[:, dc, :],
                            xt[:],
                            start=(dc == 0),
                            stop=(dc == DC - 1),
                        )
                    hi_t, lo_t = (qhT_hi, qhT_lo) if t_idx == 0 else (khT_hi, khT_lo)
                    for h in range(NH):
                        sl = slice(nb * 512, (nb + 1) * 512)
                        nc.scalar.copy(
                            hi_t[h][0:DK, sl], ps[h * DK : (h + 1) * DK, :]
                        )
                        if use_lo:
                            nc.vector.tensor_sub(
                                out=lo_t[h][0:DK, sl],
                                in0=ps[h * DK : (h + 1) * DK, :],
                                in1=hi_t[h][0:DK, sl],
                            )
                def vproj_emit(nb):
                    ps = ps_proj.tile([P, 512], F32, tag="proj", name="ps")
                    for dc in range(DC):
                        xt = stream.tile([P, 512], F32R, tag="xin", name="xtv")
                        nc.sync.dma_start(
                            xt[:],
                            vT[dc * P : (dc + 1) * P, nb * 512 : (nb + 1) * 512],
                        )
                        nc.tensor.matmul(
                            ps[:],
                            wv_sb[:, dc, :],
                            xt[:],
                            start=(dc == 0),
                            stop=(dc == DC - 1),
                        )
                    vtmp = stream.tile([P, 512], F32R, tag="xin", name="vtmp")
                    nc.vector.tensor_copy(vtmp[:], ps[:])
                    for h in range(NH):
                        pst = ps_misc.tile([P, 512], F32R, tag="misc", name="pst")
                        for j in range(4):
                            nc.tensor.transpose(
                                pst[0:P, j * DK : (j + 1) * DK],
                                vtmp[h * DK : (h + 1) * DK, j * P : (j + 1) * P],
                                ident_r[h * DK : (h + 1) * DK, h * DK : (h + 1) * DK],
                            )
                        nc.vector.tensor_copy(
                            vh[h][:, nb * 4 : nb * 4 + 4, 0:DK],
                            pst[:, 0 : 4 * DK].rearrange("p (j d) -> p j d", j=4),
                        )

                if stages < 3:
                    for t_idx in range(2):
                        for nb in range(NB):
                            proj_emit(t_idx, nb)
                    for nb in range(NB):
                        vproj_emit(nb)

                # ---- stages 2-4, interleaved emission ----
                # PE streams are in final program order: putting all of
                # stats(h1) between stats(h0) and S^T(h0) makes PE crawl at
                # DVE-reduce pace (stat-bank backpressure). Instead slice
                # stats(h1) into the S^T(h0) nb-loop so the reduces drain
                # under S^T compute, and trail W_O behind S^T(h1).
                ID = mybir.ActivationFunctionType.Identity

                SPAN_STAT = P if pad128 else DK

                def stats_emit(h, qbs):
                    for qb in qbs:
                        kmax = qb // 4 + 1 if causal else NB
                        mpart = smalls.tile([P, NB], F32, tag="mpart", name="mpart")
                        for kc in range(kmax):
                            ps = ps_stat.tile([P, 512], F32, tag="stat", name="ps_stat")
                            diag = causal and (kc == qb // 4)
                            nv = (qb % 4) * P + P if diag else 512
                            nc.tensor.matmul(
                                ps[:, 0:nv],
                                qhT_hi[h][0:SPAN_STAT, qb * P : (qb + 1) * P],
                                khT_hi[h][0:SPAN_STAT, kc * 512 : kc * 512 + nv],
                                start=True,
                                stop=not diag,
                            )
                            if diag:
                                nc.tensor.matmul(
                                    ps[:, nv - P : nv],
                                    ident_b[:],
                                    mb_sb[:],
                                    start=False,
                                    stop=True,
                                )
                            nc.vector.reduce_max(
                                mpart[:, kc : kc + 1],
                                ps[:, 0:nv],
                                axis=mybir.AxisListType.X,
                            )
                        if not use_lo:
                            # reduce straight to f32r (-m to 13 bits is a
                            # harmless common row shift) and DMA it into the
                            # rider row — no mcol_hi staging.
                            with nc.allow_low_precision(reason="-m rider, 13-bit ok"):
                                nc.vector.tensor_reduce(
                                    mcol_hi[h][:, qb : qb + 1],
                                    mpart[:, 0:kmax],
                                    axis=mybir.AxisListType.X,
                                    op=mybir.AluOpType.max,
                                    negate=True,
                                )
                            nc.sync.dma_start(
                                qhT_hi[h][DK : DK + 1, qb * P : (qb + 1) * P],
                                mcol_hi[h][:, qb : qb + 1],
                            )
                        else:
                            nc.vector.tensor_reduce(
                                mcol[h][:, qb : qb + 1],
                                mpart[:, 0:kmax],
                                axis=mybir.AxisListType.X,
                                op=mybir.AluOpType.max,
                                negate=True,
                            )
                            nc.vector.tensor_copy(
                                mcol_hi[h][:, qb : qb + 1], mcol[h][:, qb : qb + 1]
                            )
                            nc.vector.tensor_sub(
                                out=mcol_lo[h][:, qb : qb + 1],
                                in0=mcol[h][:, qb : qb + 1],
                                in1=mcol_hi[h][:, qb : qb + 1],
                            )
                            nc.sync.dma_start(
                                qhT_hi[h][DK : DK + 1, qb * P : (qb + 1) * P],
                                mcol_hi[h][:, qb : qb + 1],
                            )
                            nc.sync.dma_start(
                                qhT_lo[h][DK : DK + 1, qb * P : (qb + 1) * P],
                                mcol_lo[h][:, qb : qb + 1],
                            )

                def st3_emit(h, nb):
                    nkc = 4 * (nb + 1) if causal else QB
                    po = ps_misc.tile([P, 512], F32, tag="misc", name="po")
                    pss = {}

                    def s_mm(kc):
                        ps = ps_st.tile([P, 512], F32, tag="st", name="ps_st")
                        diag = causal and (kc >= 4 * nb)
                        o = kc - 4 * nb if diag else 0
                        qoff = o * P
                        nv = 512 - qoff
                        kslice = slice(kc * P, (kc + 1) * P)
                        qslice = slice(nb * 512 + qoff, (nb + 1) * 512)
                        span = P if pad128 else DK + 1
                        passes = [(khT_hi, qhT_hi), (khT_hi, qhT_lo), (khT_lo, qhT_hi)][
                            :st_passes
                        ]
                        for pi, (kt, qt) in enumerate(passes):
                            nc.tensor.matmul(
                                ps[:, 0:nv],
                                kt[h][0:span, kslice],
                                qt[h][0:span, qslice],
                                start=(pi == 0),
                                stop=(pi == len(passes) - 1) and not diag,
                            )
                        if diag:
                            nc.tensor.matmul(
                                ps[:, 0:P],
                                ident_b[:],
                                mf_sb[:],
                                start=False,
                                stop=True,
                            )
                        pss[kc] = (ps, qoff, nv)

                    s_mm(0)
                    for kc in range(nkc):
                        if kc + 1 < nkc:
                            s_mm(kc + 1)
                        ps, qoff, nv = pss.pop(kc)
                        pt = ptp.tile([P, 512], F32R, tag="pt", name="pt")
                        nc.scalar.activation(pt[:, 0:nv], ps[:, 0:nv], EXP)
                        nc.tensor.matmul(
                            po[0 : DK + 1, qoff:512],
                            vh[h][:, kc, :],
                            pt[:, 0:nv],
                            start=(kc == 0),
                            stop=(kc == nkc - 1),
                        )
                    nc.scalar.copy(
                        ct[h * DK : (h + 1) * DK, nb * 512 : (nb + 1) * 512],
                        po[0:DK, :],
                    )
                    lrow = 32 * h  # engine writes must start at partition 0/32/64/96
                    if wo_fuse:
                        with nc.allow_low_precision(reason="1/l scale, 13-bit ok"):
                            nc.vector.reciprocal(
                                lrT[lrow : lrow + 1, nb * 512 : (nb + 1) * 512],
                                po[DK : DK + 1, :],
                            )
                    else:
                        nc.vector.tensor_copy(
                            lscr[lrow : lrow + 1, 0:512],
                            po[DK : DK + 1, :],
                        )
                    if stages >= 4 and not wo_fuse:
                        for j in range(4):
                            qb = nb * 4 + j
                            nc.sync.dma_start(
                                lcol[h][:, qb : qb + 1],
                                lscr[lrow : lrow + 1, j * P : (j + 1) * P],
                            )
                        nc.vector.reciprocal(
                            rcol[h][:, nb * 4 : nb * 4 + 4],
                            lcol[h][:, nb * 4 : nb * 4 + 4],
                        )

                def scale_emit(nbv):
                    # ct[:, nbv*512:(nbv+1)*512] *= 1/l broadcast over head
                    # halves: fp32 rank-2 matmul sel2 x lrT -> R [128, 512]
                    # in PSUM -> one DVE mul.
                    sl = slice(nbv * 512, (nbv + 1) * 512)
                    Rps = ps_misc.tile([P, 512], F32, tag="misc", name="Rps")
                    nc.tensor.matmul(
                        Rps[:], sel2[0:33, :], lrT[0:33, sl], start=True, stop=True
                    )
                    nc.vector.tensor_mul(out=ct[:, sl], in0=ct[:, sl], in1=Rps[:])

                def wo_emit(qc):
                    if wo_fuse:
                        for eb in range(2):
                            psy = ps_proj.tile([P, 512], F32, tag="proj", name="psy")
                            nc.tensor.matmul(
                                psy[:],
                                ct[:, qc * P : (qc + 1) * P],
                                wo_sb[:, eb * 512 : (eb + 1) * 512],
                                start=True,
                                stop=True,
                            )
                            ysb = ypool.tile([P, 512], F32, tag="ysb0", name="ysb")
                            # split PSUM->SBUF copies across DVE and ACT
                            if eb == 0:
                                nc.vector.tensor_copy(ysb[:], psy[:])
                            else:
                                nc.scalar.copy(ysb[:], psy[:])
                            nc.sync.dma_start(
                                y0[qc * P : (qc + 1) * P, eb * 512 : (eb + 1) * 512],
                                ysb[:],
                            )
                        return
                    if comb_y:
                        for eb in range(2):
                            ysb0 = ypool.tile([P, 512], F32, tag="ysb0", name="ysb0")
                            ysb1 = ypool.tile([P, 512], F32, tag="ysb1", name="ysb1")
                            for h, ysb in ((0, ysb0), (1, ysb1)):
                                psy = ps_proj.tile([P, 512], F32, tag="proj", name="psy")
                                nc.tensor.matmul(
                                    psy[:],
                                    ct[h * DK : (h + 1) * DK, qc * P : (qc + 1) * P],
                                    wo_sb[h * DK : (h + 1) * DK, eb * 512 : (eb + 1) * 512],
                                    start=True,
                                    stop=True,
                                )
                                nc.scalar.activation(
                                    ysb[:], psy[:], ID, scale=rcol[h][:, qc : qc + 1]
                                )
                            nc.vector.tensor_add(out=ysb0[:], in0=ysb0[:], in1=ysb1[:])
                            nc.sync.dma_start(
                                y0[qc * P : (qc + 1) * P, eb * 512 : (eb + 1) * 512],
                                ysb0[:],
                            )
                    else:
                        for h, ydram in ((0, y0), (1, y1)):
                            ysb = ypool.tile([P, D], F32, tag="ysb", name="ysb")
                            for eb in range(2):
                                psy = ps_proj.tile([P, 512], F32, tag="proj", name="psy")
                                nc.tensor.matmul(
                                    psy[:],
                                    ct[h * DK : (h + 1) * DK, qc * P : (qc + 1) * P],
                                    wo_sb[h * DK : (h + 1) * DK, eb * 512 : (eb + 1) * 512],
                                    start=True,
                                    stop=True,
                                )
                                nc.scalar.activation(
                                    ysb[:, eb * 512 : (eb + 1) * 512],
                                    psy[:],
                                    ID,
                                    scale=rcol[h][:, qc : qc + 1],
                                )
                            nc.sync.dma_start(ydram[qc * P : (qc + 1) * P, :], ysb[:])

                if stages >= 2 and stages < 3:
                    for t_idx in range(2):
                        for nb in range(NB):
                            proj_emit(t_idx, nb)
                    stats_emit(0, range(QB))
                    stats_emit(1, range(QB))
                if stages >= 3:
                    def wo4(nb):
                        for j in range(4 if stages >= 5 else 0):
                            wo_emit(nb * 4 + j)

                    for t_idx in range(2):
                        for nb in range(NB):
                            proj_emit(t_idx, nb)
                    stats_emit(0, range(0, 4))
                    for nb in range(NB):
                        vproj_emit(nb)
                        if nb >= 2:
                            wo_emit((nb - 2) * 4 + 0)
                            wo_emit((nb - 2) * 4 + 1)
                        if nb + 1 < NB:
                            stats_emit(0, range(4 * (nb + 1), 4 * (nb + 2)))
                        st3_emit(0, nb)
                        if nb >= 2:
                            wo_emit((nb - 2) * 4 + 2)
                        stats_emit(1, range(4 * nb, 4 * nb + 4))
                        if nb >= 1:
                            st3_emit(1, nb - 1)
                            if wo_fuse:
                                scale_emit(nb - 1)
                        if nb >= 2:
                            wo_emit((nb - 2) * 4 + 3)
                    if wo_fuse:
                        # spread nbv=NB-2's blocks around the final st3 so
                        # the drain tail only owes nbv=NB-1.
                        wo_emit((NB - 2) * 4 + 0)
                        wo_emit((NB - 2) * 4 + 1)
                        st3_emit(1, NB - 1)
                        wo_emit((NB - 2) * 4 + 2)
                        wo_emit((NB - 2) * 4 + 3)
                        scale_emit(NB - 1)
                        wo4(NB - 1)
                    else:
                        st3_emit(1, NB - 1)
                        wo4(NB - 2)
                        wo4(NB - 1)

    _split_waits(nc)
    return nc


_cache = {}


def _get_nc(causal: bool):
    if causal not in _cache:
        nc = bass.Bass(trn_type="TRN2")
        build(nc, causal=causal)
        _cache[causal] = nc
    return _cache[causal]


def _host_masks():
    p = np.arange(P)[:, None]
    j = np.arange(P)[None, :]
    # S^T diag tile [kc, q]: nonzero only in the first 128 q-cols: p > j
    maskf = np.where(p > j, NEG, 0.0).astype(ml_dtypes.bfloat16)
    # stats diag tile [q, kc]: nonzero only in the last 128 kc-cols: j > p
    maskb = np.where(j > p, NEG, 0.0).astype(ml_dtypes.bfloat16)
    return maskf, maskb


LAST_EXEC_NS = None


def make_in_maps(np_inputs):
    Q = np.asarray(np_inputs["Q"], dtype=np.float32)
    K = np.asarray(np_inputs["K"], dtype=np.float32)
    V = np.asarray(np_inputs["V"], dtype=np.float32)
    W_Q = np.asarray(np_inputs["W_Q"], dtype=np.float32)
    W_K = np.asarray(np_inputs["W_K"], dtype=np.float32)
    W_V = np.asarray(np_inputs["W_V"], dtype=np.float32)
    W_O = np.asarray(np_inputs["W_O"], dtype=np.float32)

    qTh = np.ascontiguousarray(Q.T)
    kTh = np.ascontiguousarray(K.T)
    vTh = np.ascontiguousarray(V.T)
    maskf, maskb = _host_masks()
    ones_row = np.ones((1, S), dtype=np.float32)
    zeros_row = np.zeros((1, S), dtype=np.float32)
    sel2 = np.zeros((33, P), dtype=np.float32)
    sel2[0, 0:DK] = 1.0
    sel2[32, DK:P] = 1.0

    scale = np.float32(1.0 / np.sqrt(DK))
    in_maps = []
    for c in range(NCORES):
        h0, h1 = 2 * c, 2 * c + 1
        wq2 = np.ascontiguousarray(
            np.concatenate([W_Q[h0] * scale, W_Q[h1] * scale], axis=1)
        ).astype(np.float32)
        wk2 = np.ascontiguousarray(np.concatenate([W_K[h0], W_K[h1]], axis=1))
        wv2 = np.ascontiguousarray(np.concatenate([W_V[h0], W_V[h1]], axis=1))
        wo2 = np.ascontiguousarray(W_O[P * c : P * (c + 1), :])
        in_maps.append(
            {
                "qT": qTh,
                "kT": kTh,
                "vT": vTh,
                "wq": wq2,
                "wk": wk2,
                "wv": wv2,
                "wo": wo2,
                "maskf": maskf,
                "maskb": maskb,
                "onesr": ones_row,
                "zerosr": zeros_row,
                "sel2d": sel2,
                "zeros64": np.zeros((DK, S), dtype=np.float32),
            }
        )
    return in_maps


LAST_EXEC_NS = None


def kernel(Q, K, V, W_Q, W_K, W_V, W_O, mask):
    global LAST_EXEC_NS
    causal = bool(np.asarray(mask).item())
    nc = _get_nc(causal)
    in_maps = make_in_maps(
        dict(Q=Q, K=K, V=V, W_Q=W_Q, W_K=W_K, W_V=W_V, W_O=W_O)
    )

    trace = bool(int(os.environ.get("ATTN_TRACE", "0")))
    res = run_bass_kernel_spmd(
        nc, in_maps, core_ids=list(range(NCORES)), trace=trace
    )
    LAST_EXEC_NS = res.exec_time_ns

    out = np.zeros((S, D), dtype=np.float32)
    for c in range(NCORES):
        out += res.results[c]["y0"]
        if "y1" in res.results[c]:
            out += res.results[c]["y1"]
    return out



# revision 14
# speedup vs baseline: 1.0248x; 1.0248x over previous
"""Multi-head causal attention (SEQ=4096, D=1024, H=16, DK=64) on 8 TRN2
NeuronCores, tensor-parallel over heads (2 heads/core). Self-contained.

v1 pipeline (per core, 2 heads):
  One fused nb-loop (512-seq blocks) that interleaves everything so the PE
  never idles and the HAM clock gate stays at 8/8:
    per nb: Qproj(nb), Kproj(nb) | st3(h1, nb-1), scale(nb-1) | Vproj(nb) |
            stats(h0, nb), stats(h1, nb) | st3(h0, nb) | W_O(nb-1)
  - Projections: W.T @ X^T streamed from DRAM (f32r; V in bf16).
  - Stats pass: S = Qh^T.T @ Kh^T (f32r, pad-128 contraction) -> bf16 PSUM
    pairs ([P,1024] = 1 bank) -> one DVE reduce_max per pair (bf16 2x mode).
    Row max -m DMA'd into the rider row (row 64) of qhT.
  - S^T pass: [Kh;1].T @ [Qh;-m] (f32r 1-pass, the -m rider makes exp input
    bounded) -> bf16 PSUM pairs -> one ACT exp per pair -> P^T bf16 ->
    AV accumulate in f32 PSUM with ones-augmented bf16 Vh giving l as row 64.
  - 1/l via reciprocal_approx_fast (single custom-DVE op), R = rank-2
    broadcast over head halves (PE), ct scaled (DVE), Y = ct.T @ W_O_rows,
    bf16 partials DMA'd out; host sums 8 partials in fp32.
"""

import os
import sys

sys.path.insert(0, "/opt/trn_rl_repo")

import numpy as np
import ml_dtypes

import concourse.bass as bass
import concourse.mybir as mybir
import concourse.tile as tile
from concourse.bass_utils import run_bass_kernel_spmd
from concourse.masks import make_identity

P = 128
S = 4096
D = 1024
DK = 64
NH = 2  # heads per core
NCORES = 8
NEG = -1.0e9
F32 = mybir.dt.float32
F32R = mybir.dt.float32r
BF16 = mybir.dt.bfloat16
EXP = mybir.ActivationFunctionType.Exp

_ctr = [0]


def _split_waits(nc, max_waits=1):
    """walrus rejects >1 sem-wait per instruction; move extras onto
    preceding same-engine NOPs (engine streams are program-ordered)."""
    for f in nc.m.functions:
        for bb in f.blocks:
            insts = bb.instructions
            new = []
            changed = False
            for inst in insts:
                si = inst.sync_info
                if si is not None and si.on_wait and len(si.on_wait) > max_waits:
                    waits = list(si.on_wait)
                    extra, keep = waits[:-max_waits], waits[-max_waits:]
                    for i in range(0, len(extra), max_waits):
                        _ctr[0] += 1
                        new.append(
                            mybir.InstNoOp(
                                name=f"waitsplit-{_ctr[0]}",
                                engine=inst.engine,
                                ins=[],
                                outs=[],
                                sync_info=mybir.SyncInfo(
                                    on_wait=extra[i : i + max_waits], on_update=[]
                                ),
                            )
                        )
                    inst.sync_info = mybir.SyncInfo(
                        on_wait=keep, on_update=list(si.on_update)
                    )
                    changed = True
                new.append(inst)
            if changed:
                bb.instructions = new


def build(nc: bass.Bass, causal: bool = True):
    repeat = int(os.environ.get("ATTN_REPEAT", "1"))
    bf_stat = bool(int(os.environ.get("ATTN_BF16_STAT", "1")))
    bf_st = bool(int(os.environ.get("ATTN_BF16_ST", "1")))
    bf_v = bool(int(os.environ.get("ATTN_BF16_V", "1")))
    bf_y = bool(int(os.environ.get("ATTN_BF16_Y", "1")))
    recip_fast = bool(int(os.environ.get("ATTN_RECIP_FAST", "1")))
    pad128 = bool(int(os.environ.get("ATTN_PAD128", "1")))
    NB = S // 512  # 8   512-wide blocks
    QB = S // P  # 32  128-wide q blocks
    DC = D // P  # 8   128-deep contraction chunks
    FV = BF16 if bf_v else F32R  # vT/wv DMA dtype
    FP = BF16 if (bf_st or bf_v) else F32R  # pt + vh compute dtype
    FY = BF16 if bf_y else F32

    qT = nc.dram_tensor("qT", [D, S], F32R, kind="ExternalInput")
    kT = nc.dram_tensor("kT", [D, S], F32R, kind="ExternalInput")
    vT = nc.dram_tensor("vT", [D, S], FV, kind="ExternalInput")
    wq = nc.dram_tensor("wq", [D, NH * DK], F32R, kind="ExternalInput")
    wk = nc.dram_tensor("wk", [D, NH * DK], F32R, kind="ExternalInput")
    wv = nc.dram_tensor("wv", [D, NH * DK], FV, kind="ExternalInput")
    wo = nc.dram_tensor("wo", [NH * DK, D], F32R, kind="ExternalInput")
    maskf = nc.dram_tensor("maskf", [P, P], BF16, kind="ExternalInput")
    maskb = nc.dram_tensor("maskb", [P, P], BF16, kind="ExternalInput")
    onesr = nc.dram_tensor("onesr", [1, S], F32R, kind="ExternalInput")
    sel2d = nc.dram_tensor("sel2d", [33, P], F32R, kind="ExternalInput")
    zeros64 = nc.dram_tensor("zeros64", [DK, S], F32R, kind="ExternalInput")
    y0 = nc.dram_tensor("y0", [S, D], FY, kind="ExternalOutput")

    with tile.TileContext(nc) as tc:
        import contextlib

        ctx = contextlib.ExitStack()
        with ctx:
            const = ctx.enter_context(tc.tile_pool(name="const", bufs=1))
            big = ctx.enter_context(tc.tile_pool(name="big", bufs=1))
            stream = ctx.enter_context(
                tc.tile_pool(name="stream", bufs=int(os.environ.get("ATTN_BSTREAM", "8")))
            )
            ptp = ctx.enter_context(
                tc.tile_pool(name="ptp", bufs=int(os.environ.get("ATTN_BPT", "3")))
            )
            ypool = ctx.enter_context(
                tc.tile_pool(name="ypool", bufs=int(os.environ.get("ATTN_BY", "2")))
            )
            smalls = ctx.enter_context(tc.tile_pool(name="smalls", bufs=2))
            bproj = int(os.environ.get("ATTN_BPROJ", "2"))
            bstat = int(os.environ.get("ATTN_BSTAT", "2"))
            bst = int(os.environ.get("ATTN_BST", "2"))
            bmisc = int(os.environ.get("ATTN_BMISC", "2"))
            ps_proj = ctx.enter_context(
                tc.tile_pool(name="ps_proj", bufs=bproj, space="PSUM")
            )
            ps_stat = ctx.enter_context(
                tc.tile_pool(name="ps_stat", bufs=bstat, space="PSUM")
            )
            ps_st = ctx.enter_context(tc.tile_pool(name="ps_st", bufs=bst, space="PSUM"))
            ps_misc = ctx.enter_context(
                tc.tile_pool(name="ps_misc", bufs=bmisc, space="PSUM")
            )

            # ---- constants ----
            ident = const.tile([P, P], F32)
            make_identity(nc, ident[:])
            ident_b = const.tile([P, P], BF16)
            nc.vector.tensor_copy(ident_b[:], ident[:])
            ident_r = const.tile([P, P], F32R)
            nc.vector.tensor_copy(ident_r[:], ident[:])
            ident_v = ident_b if FP == BF16 else ident_r

            wq_sb = const.tile([P, DC, P], F32R, tag="wq")
            wk_sb = const.tile([P, DC, P], F32R, tag="wk")
            wv_sb = const.tile([P, DC, P], FV, tag="wv")
            nc.sync.dma_start(wq_sb[:], wq.rearrange("(o p) m -> p o m", p=P))
            nc.sync.dma_start(wk_sb[:], wk.rearrange("(o p) m -> p o m", p=P))
            nc.sync.dma_start(wv_sb[:], wv.rearrange("(o p) m -> p o m", p=P))
            wo_sb = const.tile([P, D], F32R, tag="wo")
            nc.sync.dma_start(wo_sb[:], wo[:])

            mf_sb = const.tile([P, P], BF16, tag="mf")
            mb_sb = const.tile([P, P], BF16, tag="mb")
            nc.sync.dma_start(mf_sb[:], maskf[:])
            nc.sync.dma_start(mb_sb[:], maskb[:])

            # ---- persistent activations ----
            qhT = [big.tile([P, S], F32R, tag=f"qhT{h}", name=f"qhT{h}") for h in range(NH)]
            khT = [big.tile([P, S], F32R, tag=f"khT{h}", name=f"khT{h}") for h in range(NH)]
            vh = [big.tile([P, QB, DK + 1], FP, tag=f"vh{h}", name=f"vh{h}") for h in range(NH)]
            ct = big.tile([P, S], F32R, tag="ct")
            mcol = [big.tile([P, QB], F32R, tag=f"mcol{h}", name=f"mcol{h}") for h in range(NH)]
            # 1/l rows (rows 0 / 32): consumed by the rank-2 R matmul.
            lrT = big.tile([33, S], F32R, tag="lrT")

            ones_qb = const.tile([P, QB], F32, tag="ones_qb")
            nc.any.memset(ones_qb[:], 1.0)
            # head-half selector for the rank-2 R matmul: row 0 ->
            # output partitions 0..63, row 32 -> 64..127; rows 1..31 zero.
            sel2 = const.tile([P, P], F32R, tag="sel2")
            nc.sync.dma_start(sel2[0:33, :], sel2d[:])
            # rows 1..31 of lrT ride into the R matmul as moving junk x zero
            # stationary - zero them so no NaN poisons 0*x.
            nc.scalar.dma_start(lrT[0:33, :], zeros64[0:33, :])
            for h in range(NH):
                if pad128:
                    # zero pad rows once, on the ACT-side DMA queue.
                    # (memset output trips the f32r rounding check, so DMA
                    # host zeros instead.)
                    nc.scalar.dma_start(qhT[h][DK:P, :], zeros64[:])
                    nc.scalar.dma_start(khT[h][DK:P, :], zeros64[:])
                nc.sync.dma_start(khT[h][DK : DK + 1, :], onesr[:])  # ones row
                if FP == BF16:
                    nc.gpsimd.memset(vh[h][:, :, DK], 1.0)  # ones col
                else:
                    nc.vector.tensor_copy(vh[h][:, :, DK], ones_qb[:])

            SPAN = P if pad128 else DK

            def proj_emit(t_idx, nb):
                xdram, w_sb = [(qT, wq_sb), (kT, wk_sb)][t_idx]
                ps = ps_proj.tile([P, 512], F32, tag="proj", name="ps")
                for dc in range(DC):
                    xt = stream.tile([P, 512], F32R, tag="xin", name="xt")
                    nc.sync.dma_start(
                        xt[:],
                        xdram[dc * P : (dc + 1) * P, nb * 512 : (nb + 1) * 512],
                    )
                    nc.tensor.matmul(
                        ps[:],
                        w_sb[:, dc, :],
                        xt[:],
                        start=(dc == 0),
                        stop=(dc == DC - 1),
                    )
                hi_t = qhT if t_idx == 0 else khT
                for h in range(NH):
                    sl = slice(nb * 512, (nb + 1) * 512)
                    nc.scalar.copy(hi_t[h][0:DK, sl], ps[h * DK : (h + 1) * DK, :])

            def vproj_emit(nb):
                ps = ps_proj.tile([P, 512], F32, tag="proj", name="ps")
                for dc in range(DC):
                    xt = stream.tile([P, 512], FV, tag="xin", name="xtv")
                    nc.sync.dma_start(
                        xt[:],
                        vT[dc * P : (dc + 1) * P, nb * 512 : (nb + 1) * 512],
                    )
                    nc.tensor.matmul(
                        ps[:],
                        wv_sb[:, dc, :],
                        xt[:],
                        start=(dc == 0),
                        stop=(dc == DC - 1),
                    )
                vtmp = stream.tile([P, 512], FP, tag="xin", name="vtmp")
                nc.vector.tensor_copy(vtmp[:], ps[:])
                for h in range(NH):
                    pst = ps_misc.tile([P, 512], FP, tag="misc", name="pst")
                    for j in range(4):
                        nc.tensor.transpose(
                            pst[0:P, j * DK : (j + 1) * DK],
                            vtmp[h * DK : (h + 1) * DK, j * P : (j + 1) * P],
                            ident_v[h * DK : (h + 1) * DK, h * DK : (h + 1) * DK],
                        )
                    nc.vector.tensor_copy(
                        vh[h][:, nb * 4 : nb * 4 + 4, 0:DK],
                        pst[:, 0 : 4 * DK].rearrange("p (j d) -> p j d", j=4),
                    )

            # ---- stats: S blocks in (possibly bf16) PSUM pairs, row-max ----
            def stats_emit(h, qbs):
                for qb in qbs:
                    kmax = qb // 4 + 1 if causal else NB
                    mpart = smalls.tile([P, NB], F32, tag="mpart", name="mpart")
                    for kc in range(kmax):
                        ps = ps_stat.tile([P, 512], F32, tag="stat", name="ps_stat")
                        diag = causal and (kc == qb // 4)
                        nv = (qb % 4) * P + P if diag else 512
                        nc.tensor.matmul(
                            ps[:, 0:nv],
                            qhT[h][0:SPAN, qb * P : (qb + 1) * P],
                            khT[h][0:SPAN, kc * 512 : kc * 512 + nv],
                            start=True,
                            stop=not diag,
                        )
                        if diag:
                            nc.tensor.matmul(
                                ps[:, nv - P : nv],
                                ident_b[:],
                                mb_sb[:],
                                start=False,
                                stop=True,
                            )
                        nc.vector.reduce_max(
                            mpart[:, kc : kc + 1],
                            ps[:, 0:nv],
                            axis=mybir.AxisListType.X,
                        )
                    # reduce straight to f32r (-m to 13 bits is a harmless
                    # common row shift).
                    with nc.allow_low_precision(reason="-m rider, 13-bit ok"):
                        nc.vector.tensor_reduce(
                            mcol[h][:, qb : qb + 1],
                            mpart[:, 0:kmax],
                            axis=mybir.AxisListType.X,
                            op=mybir.AluOpType.max,
                            negate=True,
                        )
                    nc.sync.dma_start(
                        qhT[h][DK : DK + 1, qb * P : (qb + 1) * P],
                        mcol[h][:, qb : qb + 1],
                    )

            # ---- S^T + exp + AV for one (head, 512-q block) ----
            def st3_emit(h, nb):
                nkc = 4 * (nb + 1) if causal else QB
                po = ps_misc.tile([P, 512], F32, tag="misc", name="po")
                pss = {}

                def s_mm(kc):
                    ps = ps_st.tile([P, 512], F32, tag="st", name="ps_st")
                    diag = causal and (kc >= 4 * nb)
                    o = kc - 4 * nb if diag else 0
                    qoff = o * P
                    nv = 512 - qoff
                    kslice = slice(kc * P, (kc + 1) * P)
                    qslice = slice(nb * 512 + qoff, (nb + 1) * 512)
                    span = P if pad128 else DK + 1
                    nc.tensor.matmul(
                        ps[:, 0:nv],
                        khT[h][0:span, kslice],
                        qhT[h][0:span, qslice],
                        start=True,
                        stop=not diag,
                    )
                    if diag:
                        nc.tensor.matmul(
                            ps[:, 0:P],
                            ident_b[:],
                            mf_sb[:],
                            start=False,
                            stop=True,
                        )
                    pss[kc] = (ps, qoff, nv)

                s_mm(0)
                for kc in range(nkc):
                    if kc + 1 < nkc:
                        s_mm(kc + 1)
                    ps, qoff, nv = pss.pop(kc)
                    pt = ptp.tile([P, 512], FP, tag="pt", name="pt")
                    nc.scalar.activation(pt[:, 0:nv], ps[:, 0:nv], EXP)
                    nc.tensor.matmul(
                        po[0 : DK + 1, qoff:512],
                        vh[h][:, kc, :],
                        pt[:, 0:nv],
                        start=(kc == 0),
                        stop=(kc == nkc - 1),
                    )
                nc.scalar.copy(
                    ct[h * DK : (h + 1) * DK, nb * 512 : (nb + 1) * 512],
                    po[0:DK, :],
                )
                lrow = 32 * h
                if recip_fast:
                    # lrT holds plain l; scale_emit broadcasts it and takes
                    # the reciprocal there in a well-shaped [128,512] op.
                    nc.scalar.copy(
                        lrT[lrow : lrow + 1, nb * 512 : (nb + 1) * 512],
                        po[DK : DK + 1, :],
                    )
                else:
                    with nc.allow_low_precision(reason="1/l scale"):
                        nc.vector.reciprocal(
                            lrT[lrow : lrow + 1, nb * 512 : (nb + 1) * 512],
                            po[DK : DK + 1, :],
                        )

            def scale_emit(nbv):
                # ct[:, nbv*512:(nbv+1)*512] *= 1/l broadcast over head
                # halves: rank-2 matmul sel2 x lrT -> [128, 512] in PSUM
                # (l or 1/l per recip_fast) -> recip (if needed) -> DVE mul.
                sl = slice(nbv * 512, (nbv + 1) * 512)
                Rps = ps_misc.tile([P, 512], F32, tag="misc", name="Rps")
                nc.tensor.matmul(
                    Rps[:], sel2[0:33, :], lrT[0:33, sl], start=True, stop=True
                )
                if recip_fast:
                    # one [128,512] reciprocal on the broadcast (vs two
                    # [1,512] rows), then the scale mul.
                    rin = smalls.tile([P, 512], F32, tag="rinv", name="rinv")
                    nc.vector.reciprocal(rin[:], Rps[:])
                    nc.vector.tensor_mul(out=ct[:, sl], in0=ct[:, sl], in1=rin[:])
                else:
                    nc.vector.tensor_mul(out=ct[:, sl], in0=ct[:, sl], in1=Rps[:])

            def wo_emit(qc):
                ysb = ypool.tile([P, D], FY, tag="ysb", name="ysb")
                for eb in range(2):
                    psy = ps_proj.tile([P, 512], F32, tag="proj", name="psy")
                    nc.tensor.matmul(
                        psy[:],
                        ct[:, qc * P : (qc + 1) * P],
                        wo_sb[:, eb * 512 : (eb + 1) * 512],
                        start=True,
                        stop=True,
                    )
                    # split PSUM->SBUF copies across DVE and ACT
                    if eb == 0:
                        nc.vector.tensor_copy(ysb[:, 0:512], psy[:])
                    else:
                        nc.scalar.copy(ysb[:, 512:1024], psy[:])
                nc.sync.dma_start(y0[qc * P : (qc + 1) * P, :], ysb[:])

            def wo4(nb):
                for j in range(4):
                    wo_emit(nb * 4 + j)

            for _rep in range(repeat):
                for nb in range(NB):
                    proj_emit(0, nb)
                    proj_emit(1, nb)
                    if nb >= 1:
                        st3_emit(1, nb - 1)
                        scale_emit(nb - 1)
                    vproj_emit(nb)
                    stats_emit(0, range(4 * nb, 4 * nb + 4))
                    stats_emit(1, range(4 * nb, 4 * nb + 4))
                    st3_emit(0, nb)
                    if nb >= 1:
                        wo4(nb - 1)
                st3_emit(1, NB - 1)
                scale_emit(NB - 1)
                wo4(NB - 1)

    _split_waits(nc)
    return nc


_cache = {}


def _get_nc(causal: bool):
    if causal not in _cache:
        nc = bass.Bass(trn_type="TRN2")
        build(nc, causal=causal)
        _cache[causal] = nc
    return _cache[causal]


def _host_masks():
    p = np.arange(P)[:, None]
    j = np.arange(P)[None, :]
    # S^T diag tile [kc, q]: nonzero only in the first 128 q-cols: p > j
    maskf = np.where(p > j, NEG, 0.0).astype(ml_dtypes.bfloat16)
    # stats diag tile [q, kc]: nonzero only in the last 128 kc-cols: j > p
    maskb = np.where(j > p, NEG, 0.0).astype(ml_dtypes.bfloat16)
    return maskf, maskb


def make_in_maps(np_inputs):
    bf_v = bool(int(os.environ.get("ATTN_BF16_V", "1")))
    Q = np.asarray(np_inputs["Q"], dtype=np.float32)
    K = np.asarray(np_inputs["K"], dtype=np.float32)
    V = np.asarray(np_inputs["V"], dtype=np.float32)
    W_Q = np.asarray(np_inputs["W_Q"], dtype=np.float32)
    W_K = np.asarray(np_inputs["W_K"], dtype=np.float32)
    W_V = np.asarray(np_inputs["W_V"], dtype=np.float32)
    W_O = np.asarray(np_inputs["W_O"], dtype=np.float32)

    qTh = np.ascontiguousarray(Q.T)
    kTh = np.ascontiguousarray(K.T)
    vTh = np.ascontiguousarray(V.T)
    if bf_v:
        vTh = vTh.astype(ml_dtypes.bfloat16)
    maskf, maskb = _host_masks()
    ones_row = np.ones((1, S), dtype=np.float32)
    sel2 = np.zeros((33, P), dtype=np.float32)
    sel2[0, 0:DK] = 1.0
    sel2[32, DK:P] = 1.0

    scale = np.float32(1.0 / np.sqrt(DK))
    in_maps = []
    for c in range(NCORES):
        h0, h1 = 2 * c, 2 * c + 1
        wq2 = np.ascontiguousarray(
            np.concatenate([W_Q[h0] * scale, W_Q[h1] * scale], axis=1)
        ).astype(np.float32)
        wk2 = np.ascontiguousarray(np.concatenate([W_K[h0], W_K[h1]], axis=1))
        wv2 = np.ascontiguousarray(np.concatenate([W_V[h0], W_V[h1]], axis=1))
        if bf_v:
            wv2 = wv2.astype(ml_dtypes.bfloat16)
        wo2 = np.ascontiguousarray(W_O[P * c : P * (c + 1), :])
        in_maps.append(
            {
                "qT": qTh,
                "kT": kTh,
                "vT": vTh,
                "wq": wq2,
                "wk": wk2,
                "wv": wv2,
                "wo": wo2,
                "maskf": maskf,
                "maskb": maskb,
                "onesr": ones_row,
                "sel2d": sel2,
                "zeros64": np.zeros((DK, S), dtype=np.float32),
            }
        )
    return in_maps


LAST_EXEC_NS = None


def kernel(Q, K, V, W_Q, W_K, W_V, W_O, mask):
    global LAST_EXEC_NS
    causal = bool(np.asarray(mask).item())
    nc = _get_nc(causal)
    in_maps = make_in_maps(
        dict(Q=Q, K=K, V=V, W_Q=W_Q, W_K=W_K, W_V=W_V, W_O=W_O)
    )

    trace = bool(int(os.environ.get("ATTN_TRACE", "0")))
    res = run_bass_kernel_spmd(
        nc, in_maps, core_ids=list(range(NCORES)), trace=trace
    )
    LAST_EXEC_NS = res.exec_time_ns

    out = np.zeros((S, D), dtype=np.float32)
    for c in range(NCORES):
        out += np.asarray(res.results[c]["y0"], dtype=np.float32)
    return out


# revision 20
# speedup vs baseline: 1.1472x; 1.1194x over previous
"""Multi-head causal attention (SEQ=4096, D=1024, H=16, DK=64) on 8 TRN2
NeuronCores, tensor-parallel over heads (2 heads/core). Self-contained.

v1 pipeline (per core, 2 heads):
  One fused nb-loop (512-seq blocks) that interleaves everything so the PE
  never idles and the HAM clock gate stays at 8/8:
    per nb: Qproj(nb), Kproj(nb) | st3(h1, nb-1), scale(nb-1) | Vproj(nb) |
            stats(h0, nb), stats(h1, nb) | st3(h0, nb) | W_O(nb-1)
  - Projections: W.T @ X^T streamed from DRAM (f32r; V in bf16).
  - Stats pass: S = Qh^T.T @ Kh^T (f32r, pad-128 contraction) -> bf16 PSUM
    pairs ([P,1024] = 1 bank) -> one DVE reduce_max per pair (bf16 2x mode).
    Row max -m DMA'd into the rider row (row 64) of qhT.
  - S^T pass: [Kh;1].T @ [Qh;-m] (f32r 1-pass, the -m rider makes exp input
    bounded) -> bf16 PSUM pairs -> one ACT exp per pair -> P^T bf16 ->
    AV accumulate in f32 PSUM with ones-augmented bf16 Vh giving l as row 64.
  - 1/l via reciprocal_approx_fast (single custom-DVE op), R = rank-2
    broadcast over head halves (PE), ct scaled (DVE), Y = ct.T @ W_O_rows,
    bf16 partials DMA'd out; host sums 8 partials in fp32.
"""

import os
import sys

sys.path.insert(0, "/opt/trn_rl_repo")

import numpy as np
import ml_dtypes

import concourse.bass as bass
import concourse.mybir as mybir
import concourse.tile as tile
from concourse.bass_utils import run_bass_kernel_spmd
from concourse.masks import make_identity

P = 128
S = 4096
D = 1024
DK = 64
NH = 2  # heads per core
NCORES = 8
NEG = -1.0e9
F32 = mybir.dt.float32
F32R = mybir.dt.float32r
BF16 = mybir.dt.bfloat16
EXP = mybir.ActivationFunctionType.Exp

_ctr = [0]


def _split_waits(nc, max_waits=1):
    """walrus rejects >1 sem-wait per instruction; move extras onto
    preceding same-engine NOPs (engine streams are program-ordered)."""
    for f in nc.m.functions:
        for bb in f.blocks:
            insts = bb.instructions
            new = []
            changed = False
            for inst in insts:
                si = inst.sync_info
                if si is not None and si.on_wait and len(si.on_wait) > max_waits:
                    waits = list(si.on_wait)
                    extra, keep = waits[:-max_waits], waits[-max_waits:]
                    for i in range(0, len(extra), max_waits):
                        _ctr[0] += 1
                        new.append(
                            mybir.InstNoOp(
                                name=f"waitsplit-{_ctr[0]}",
                                engine=inst.engine,
                                ins=[],
                                outs=[],
                                sync_info=mybir.SyncInfo(
                                    on_wait=extra[i : i + max_waits], on_update=[]
                                ),
                            )
                        )
                    inst.sync_info = mybir.SyncInfo(
                        on_wait=keep, on_update=list(si.on_update)
                    )
                    changed = True
                new.append(inst)
            if changed:
                bb.instructions = new


def build(nc: bass.Bass, causal: bool = True):
    repeat = int(os.environ.get("ATTN_REPEAT", "1"))
    bf_stat = bool(int(os.environ.get("ATTN_BF16_STAT", "1")))
    bf_st = bool(int(os.environ.get("ATTN_BF16_ST", "1")))
    bf_v = bool(int(os.environ.get("ATTN_BF16_V", "1")))
    bf_y = bool(int(os.environ.get("ATTN_BF16_Y", "1")))
    recip_fast = bool(int(os.environ.get("ATTN_RECIP_FAST", "1")))
    pad128 = bool(int(os.environ.get("ATTN_PAD128", "1")))
    NB = S // 512  # 8   512-wide blocks
    QB = S // P  # 32  128-wide q blocks
    DC = D // P  # 8   128-deep contraction chunks
    FV = BF16 if bf_v else F32R  # vT/wv DMA dtype
    FP = BF16 if (bf_st or bf_v) else F32R  # pt + vh compute dtype
    FY = BF16 if bf_y else F32

    qT = nc.dram_tensor("qT", [D, S], F32R, kind="ExternalInput")
    kT = nc.dram_tensor("kT", [D, S], F32R, kind="ExternalInput")
    vT = nc.dram_tensor("vT", [D, S], FV, kind="ExternalInput")
    wq = nc.dram_tensor("wq", [D, NH * DK], F32R, kind="ExternalInput")
    wk = nc.dram_tensor("wk", [D, NH * DK], F32R, kind="ExternalInput")
    wv = nc.dram_tensor("wv", [D, NH * DK], FV, kind="ExternalInput")
    wo = nc.dram_tensor("wo", [NH * DK, D], F32R, kind="ExternalInput")
    maskf = nc.dram_tensor("maskf", [P, P], BF16, kind="ExternalInput")
    maskb = nc.dram_tensor("maskb", [P, P], BF16, kind="ExternalInput")
    onesr = nc.dram_tensor("onesr", [1, S], F32R, kind="ExternalInput")
    sel2d = nc.dram_tensor("sel2d", [33, P], F32R, kind="ExternalInput")
    zeros64 = nc.dram_tensor("zeros64", [DK, S], F32R, kind="ExternalInput")
    y0 = nc.dram_tensor("y0", [S, D], FY, kind="ExternalOutput")

    with tile.TileContext(nc) as tc:
        import contextlib

        ctx = contextlib.ExitStack()
        with ctx:
            const = ctx.enter_context(tc.tile_pool(name="const", bufs=1))
            big = ctx.enter_context(tc.tile_pool(name="big", bufs=1))
            stream = ctx.enter_context(
                tc.tile_pool(name="stream", bufs=int(os.environ.get("ATTN_BSTREAM", "8")))
            )
            ptp = ctx.enter_context(
                tc.tile_pool(name="ptp", bufs=int(os.environ.get("ATTN_BPT", "3")))
            )
            ypool = ctx.enter_context(
                tc.tile_pool(name="ypool", bufs=int(os.environ.get("ATTN_BY", "2")))
            )
            smalls = ctx.enter_context(tc.tile_pool(name="smalls", bufs=2))
            bproj = int(os.environ.get("ATTN_BPROJ", "2"))
            bstat = int(os.environ.get("ATTN_BSTAT", "2"))
            bst = int(os.environ.get("ATTN_BST", "3"))
            bmisc = int(os.environ.get("ATTN_BMISC", "1"))
            ps_proj = ctx.enter_context(
                tc.tile_pool(name="ps_proj", bufs=bproj, space="PSUM")
            )
            ps_stat = ctx.enter_context(
                tc.tile_pool(name="ps_stat", bufs=bstat, space="PSUM")
            )
            ps_st = ctx.enter_context(tc.tile_pool(name="ps_st", bufs=bst, space="PSUM"))
            ps_misc = ctx.enter_context(
                tc.tile_pool(name="ps_misc", bufs=bmisc, space="PSUM")
            )

            # ---- constants ----
            ident = const.tile([P, P], F32)
            make_identity(nc, ident[:])
            ident_b = const.tile([P, P], BF16)
            nc.vector.tensor_copy(ident_b[:], ident[:])
            ident_r = const.tile([P, P], F32R)
            nc.vector.tensor_copy(ident_r[:], ident[:])
            ident_v = ident_b if FP == BF16 else ident_r

            # wq/wk first on the sync (input-stream) queue so the first proj
            # matmuls start ASAP; everything else on the ACT-side queue.
            wq_sb = const.tile([P, DC, P], F32R, tag="wq")
            wk_sb = const.tile([P, DC, P], F32R, tag="wk")
            wv_sb = const.tile([P, DC, P], FV, tag="wv")
            nc.sync.dma_start(wq_sb[:], wq.rearrange("(o p) m -> p o m", p=P))
            nc.sync.dma_start(wk_sb[:], wk.rearrange("(o p) m -> p o m", p=P))
            nc.scalar.dma_start(wv_sb[:], wv.rearrange("(o p) m -> p o m", p=P))
            wo_sb = const.tile([P, D], F32R, tag="wo")
            nc.scalar.dma_start(wo_sb[:], wo[:])

            mf_sb = const.tile([P, P], BF16, tag="mf")
            mb_sb = const.tile([P, P], BF16, tag="mb")
            nc.scalar.dma_start(mf_sb[:], maskf[:])
            nc.scalar.dma_start(mb_sb[:], maskb[:])

            # ---- persistent activations ----
            qhT = [big.tile([P, S], F32R, tag=f"qhT{h}", name=f"qhT{h}") for h in range(NH)]
            khT = [big.tile([P, S], F32R, tag=f"khT{h}", name=f"khT{h}") for h in range(NH)]
            vh = [big.tile([P, QB, DK + 1], FP, tag=f"vh{h}", name=f"vh{h}") for h in range(NH)]
            ct = big.tile([P, S], F32R, tag="ct")
            mcol = [big.tile([P, QB], F32R, tag=f"mcol{h}", name=f"mcol{h}") for h in range(NH)]
            # 1/l rows (rows 0 / 32): consumed by the rank-2 R matmul.
            lrT = big.tile([33, S], F32R, tag="lrT")

            ones_qb = const.tile([P, QB], F32, tag="ones_qb")
            nc.any.memset(ones_qb[:], 1.0)
            # head-half selector for the rank-2 R matmul: row 0 ->
            # output partitions 0..63, row 32 -> 64..127; rows 1..31 zero.
            sel2 = const.tile([P, P], F32R, tag="sel2")
            nc.scalar.dma_start(sel2[0:33, :], sel2d[:])
            # rows 1..31 of lrT ride into the R matmul as moving junk x zero
            # stationary - zero them so no NaN poisons 0*x.
            nc.scalar.dma_start(lrT[0:33, :], zeros64[0:33, :])
            for h in range(NH):
                if pad128:
                    # zero pad rows once, on the ACT-side DMA queue.
                    # (memset output trips the f32r rounding check, so DMA
                    # host zeros instead.)
                    nc.scalar.dma_start(qhT[h][DK:P, :], zeros64[:])
                    nc.scalar.dma_start(khT[h][DK:P, :], zeros64[:])
                nc.sync.dma_start(khT[h][DK : DK + 1, :], onesr[:])  # ones row
                if FP == BF16:
                    nc.gpsimd.memset(vh[h][:, :, DK], 1.0)  # ones col
                else:
                    nc.vector.tensor_copy(vh[h][:, :, DK], ones_qb[:])

            SPAN = P if pad128 else DK

            def proj_emit(t_idx, nb):
                xdram, w_sb = [(qT, wq_sb), (kT, wk_sb)][t_idx]
                ps = ps_proj.tile([P, 512], F32, tag="proj", name="ps")
                for dc in range(DC):
                    xt = stream.tile([P, 512], F32R, tag="xin", name="xt")
                    nc.sync.dma_start(
                        xt[:],
                        xdram[dc * P : (dc + 1) * P, nb * 512 : (nb + 1) * 512],
                    )
                    nc.tensor.matmul(
                        ps[:],
                        w_sb[:, dc, :],
                        xt[:],
                        start=(dc == 0),
                        stop=(dc == DC - 1),
                    )
                hi_t = qhT if t_idx == 0 else khT
                for h in range(NH):
                    sl = slice(nb * 512, (nb + 1) * 512)
                    nc.scalar.copy(hi_t[h][0:DK, sl], ps[h * DK : (h + 1) * DK, :])

            def vproj_emit(nb):
                ps = ps_proj.tile([P, 512], F32, tag="proj", name="ps")
                for dc in range(DC):
                    xt = stream.tile([P, 512], FV, tag="xin", name="xtv")
                    nc.sync.dma_start(
                        xt[:],
                        vT[dc * P : (dc + 1) * P, nb * 512 : (nb + 1) * 512],
                    )
                    nc.tensor.matmul(
                        ps[:],
                        wv_sb[:, dc, :],
                        xt[:],
                        start=(dc == 0),
                        stop=(dc == DC - 1),
                    )
                vtmp = stream.tile([P, 512], FP, tag="xin", name="vtmp")
                nc.vector.tensor_copy(vtmp[:], ps[:])
                for h in range(NH):
                    pst = ps_misc.tile([P, 512], FP, tag="misc", name="pst")
                    for j in range(4):
                        nc.tensor.transpose(
                            pst[0:P, j * DK : (j + 1) * DK],
                            vtmp[h * DK : (h + 1) * DK, j * P : (j + 1) * P],
                            ident_v[h * DK : (h + 1) * DK, h * DK : (h + 1) * DK],
                        )
                    nc.vector.tensor_copy(
                        vh[h][:, nb * 4 : nb * 4 + 4, 0:DK],
                        pst[:, 0 : 4 * DK].rearrange("p (j d) -> p j d", j=4),
                    )

            # ---- stats: one S block + row-max reduce per unit (weavable) ----
            def stats_units(h, qbs):
                """Yield closures, each emitting ~1 PE matmul of stats work."""
                for qb in qbs:
                    kmax = qb // 4 + 1 if causal else NB
                    state = {}

                    def mk(kc, h=h, qb=qb, kmax=kmax, state=state):
                        def emit():
                            if kc == 0:
                                state["mpart"] = smalls.tile(
                                    [P, NB], F32, tag="mpart", name="mpart"
                                )
                            mpart = state["mpart"]
                            ps = ps_stat.tile([P, 512], F32, tag="stat", name="ps_stat")
                            diag = causal and (kc == qb // 4)
                            nv = (qb % 4) * P + P if diag else 512
                            nc.tensor.matmul(
                                ps[:, 0:nv],
                                qhT[h][0:SPAN, qb * P : (qb + 1) * P],
                                khT[h][0:SPAN, kc * 512 : kc * 512 + nv],
                                start=True,
                                stop=not diag,
                            )
                            if diag:
                                nc.tensor.matmul(
                                    ps[:, nv - P : nv],
                                    ident_b[:],
                                    mb_sb[:],
                                    start=False,
                                    stop=True,
                                )
                            nc.vector.reduce_max(
                                mpart[:, kc : kc + 1],
                                ps[:, 0:nv],
                                axis=mybir.AxisListType.X,
                            )
                            if kc == kmax - 1:
                                # reduce straight to f32r (-m to 13 bits is a
                                # harmless common row shift).
                                with nc.allow_low_precision(reason="-m rider"):
                                    nc.vector.tensor_reduce(
                                        mcol[h][:, qb : qb + 1],
                                        mpart[:, 0:kmax],
                                        axis=mybir.AxisListType.X,
                                        op=mybir.AluOpType.max,
                                        negate=True,
                                    )
                                nc.sync.dma_start(
                                    qhT[h][DK : DK + 1, qb * P : (qb + 1) * P],
                                    mcol[h][:, qb : qb + 1],
                                )

                        return emit

                    for kc in range(kmax):
                        yield mk(kc)

            # ---- S^T + exp + AV for one (head, 512-q block) ----
            # fillers: independent ~1-matmul closures woven between the
            # S^T matmul of block kc+1 and the AV of block kc so the PE
            # stays busy during the (longer) ACT exp of block kc.
            def st3_emit(h, nb, fillers=()):
                nkc = 4 * (nb + 1) if causal else QB
                fillers = list(fillers)
                nf = len(fillers)
                fi = 0
                po = ps_misc.tile([P, 512], F32, tag="misc", name="po")
                pss = {}

                def s_mm(kc):
                    ps = ps_st.tile([P, 512], F32, tag="st", name="ps_st")
                    diag = causal and (kc >= 4 * nb)
                    o = kc - 4 * nb if diag else 0
                    qoff = o * P
                    nv = 512 - qoff
                    kslice = slice(kc * P, (kc + 1) * P)
                    qslice = slice(nb * 512 + qoff, (nb + 1) * 512)
                    span = P if pad128 else DK + 1
                    nc.tensor.matmul(
                        ps[:, 0:nv],
                        khT[h][0:span, kslice],
                        qhT[h][0:span, qslice],
                        start=True,
                        stop=not diag,
                    )
                    if diag:
                        nc.tensor.matmul(
                            ps[:, 0:P],
                            ident_b[:],
                            mf_sb[:],
                            start=False,
                            stop=True,
                        )
                    pss[kc] = (ps, qoff, nv)

                s_mm(0)
                for kc in range(nkc):
                    if kc + 1 < nkc:
                        s_mm(kc + 1)
                    ps, qoff, nv = pss.pop(kc)
                    pt = ptp.tile([P, 512], FP, tag="pt", name="pt")
                    nc.scalar.activation(pt[:, 0:nv], ps[:, 0:nv], EXP)
                    # evenly spread the fillers across the kc slots
                    want = (kc + 1) * nf // nkc
                    while fi < want:
                        fillers[fi]()
                        fi += 1
                    nc.tensor.matmul(
                        po[0 : DK + 1, qoff:512],
                        vh[h][:, kc, :],
                        pt[:, 0:nv],
                        start=(kc == 0),
                        stop=(kc == nkc - 1),
                    )
                nc.scalar.copy(
                    ct[h * DK : (h + 1) * DK, nb * 512 : (nb + 1) * 512],
                    po[0:DK, :],
                )
                lrow = 32 * h
                if recip_fast:
                    # lrT holds plain l; scale_emit broadcasts it and takes
                    # the reciprocal there in a well-shaped [128,512] op.
                    nc.scalar.copy(
                        lrT[lrow : lrow + 1, nb * 512 : (nb + 1) * 512],
                        po[DK : DK + 1, :],
                    )
                else:
                    with nc.allow_low_precision(reason="1/l scale"):
                        nc.vector.reciprocal(
                            lrT[lrow : lrow + 1, nb * 512 : (nb + 1) * 512],
                            po[DK : DK + 1, :],
                        )

            def scale_emit(nbv):
                # ct[:, nbv*512:(nbv+1)*512] *= 1/l broadcast over head
                # halves: rank-2 matmul sel2 x lrT -> [128, 512] in PSUM
                # (l or 1/l per recip_fast) -> recip (if needed) -> DVE mul.
                sl = slice(nbv * 512, (nbv + 1) * 512)
                Rps = ps_misc.tile([P, 512], F32, tag="misc", name="Rps")
                nc.tensor.matmul(
                    Rps[:], sel2[0:33, :], lrT[0:33, sl], start=True, stop=True
                )
                if recip_fast:
                    # one [128,512] reciprocal on the broadcast (vs two
                    # [1,512] rows), then the scale mul.
                    rin = smalls.tile([P, 512], F32, tag="rinv", name="rinv")
                    nc.vector.reciprocal(rin[:], Rps[:])
                    nc.vector.tensor_mul(out=ct[:, sl], in0=ct[:, sl], in1=rin[:])
                else:
                    nc.vector.tensor_mul(out=ct[:, sl], in0=ct[:, sl], in1=Rps[:])

            def wo_units(nbv):
                """Yield 8 closures (one matmul each) for the 4 q-chunks of
                512-q block nbv; ct[:, nbv] must already be scaled."""
                for j in range(4):
                    qc = nbv * 4 + j
                    state = {}

                    def mk(eb, qc=qc, state=state):
                        def emit():
                            if eb == 0:
                                state["ysb"] = ypool.tile(
                                    [P, D], FY, tag="ysb", name="ysb"
                                )
                            ysb = state["ysb"]
                            psy = ps_proj.tile([P, 512], F32, tag="proj", name="psy")
                            nc.tensor.matmul(
                                psy[:],
                                ct[:, qc * P : (qc + 1) * P],
                                wo_sb[:, eb * 512 : (eb + 1) * 512],
                                start=True,
                                stop=True,
                            )
                            # split PSUM->SBUF copies across DVE and ACT
                            if eb == 0:
                                nc.vector.tensor_copy(ysb[:, 0:512], psy[:])
                            else:
                                nc.scalar.copy(ysb[:, 512:1024], psy[:])
                                nc.sync.dma_start(
                                    y0[qc * P : (qc + 1) * P, :], ysb[:]
                                )

                        return emit

                    yield mk(0)
                    yield mk(1)

            for _rep in range(repeat):
                for nb in range(NB):
                    proj_emit(0, nb)
                    proj_emit(1, nb)
                    vproj_emit(nb)
                    if nb == 0:
                        for u in stats_units(0, range(0, 4)):
                            u()
                        st3_emit(0, 0, fillers=stats_units(1, range(0, 4)))
                    else:
                        fl = []
                        if nb >= 2:
                            fl += list(wo_units(nb - 2))
                        fl += list(stats_units(0, range(4 * nb, 4 * nb + 4)))
                        st3_emit(1, nb - 1, fillers=fl)
                        scale_emit(nb - 1)
                        st3_emit(
                            0, nb, fillers=stats_units(1, range(4 * nb, 4 * nb + 4))
                        )
                st3_emit(1, NB - 1, fillers=wo_units(NB - 2))
                scale_emit(NB - 1)
                for u in wo_units(NB - 1):
                    u()

    _split_waits(nc)
    return nc


_cache = {}


def _get_nc(causal: bool):
    if causal not in _cache:
        nc = bass.Bass(trn_type="TRN2")
        build(nc, causal=causal)
        _cache[causal] = nc
    return _cache[causal]


def _host_masks():
    p = np.arange(P)[:, None]
    j = np.arange(P)[None, :]
    # S^T diag tile [kc, q]: nonzero only in the first 128 q-cols: p > j
    maskf = np.where(p > j, NEG, 0.0).astype(ml_dtypes.bfloat16)
    # stats diag tile [q, kc]: nonzero only in the last 128 kc-cols: j > p
    maskb = np.where(j > p, NEG, 0.0).astype(ml_dtypes.bfloat16)
    return maskf, maskb


def make_in_maps(np_inputs):
    bf_v = bool(int(os.environ.get("ATTN_BF16_V", "1")))
    Q = np.asarray(np_inputs["Q"], dtype=np.float32)
    K = np.asarray(np_inputs["K"], dtype=np.float32)
    V = np.asarray(np_inputs["V"], dtype=np.float32)
    W_Q = np.asarray(np_inputs["W_Q"], dtype=np.float32)
    W_K = np.asarray(np_inputs["W_K"], dtype=np.float32)
    W_V = np.asarray(np_inputs["W_V"], dtype=np.float32)
    W_O = np.asarray(np_inputs["W_O"], dtype=np.float32)

    qTh = np.ascontiguousarray(Q.T)
    kTh = np.ascontiguousarray(K.T)
    vTh = np.ascontiguousarray(V.T)
    if bf_v:
        vTh = vTh.astype(ml_dtypes.bfloat16)
    maskf, maskb = _host_masks()
    ones_row = np.ones((1, S), dtype=np.float32)
    sel2 = np.zeros((33, P), dtype=np.float32)
    sel2[0, 0:DK] = 1.0
    sel2[32, DK:P] = 1.0

    scale = np.float32(1.0 / np.sqrt(DK))
    in_maps = []
    for c in range(NCORES):
        h0, h1 = 2 * c, 2 * c + 1
        wq2 = np.ascontiguousarray(
            np.concatenate([W_Q[h0] * scale, W_Q[h1] * scale], axis=1)
        ).astype(np.float32)
        wk2 = np.ascontiguousarray(np.concatenate([W_K[h0], W_K[h1]], axis=1))
        wv2 = np.ascontiguousarray(np.concatenate([W_V[h0], W_V[h1]], axis=1))
        if bf_v:
            wv2 = wv2.astype(ml_dtypes.bfloat16)
        wo2 = np.ascontiguousarray(W_O[P * c : P * (c + 1), :])
        in_maps.append(
            {
                "qT": qTh,
                "kT": kTh,
                "vT": vTh,
                "wq": wq2,
                "wk": wk2,
                "wv": wv2,
                "wo": wo2,
                "maskf": maskf,
                "maskb": maskb,
                "onesr": ones_row,
                "sel2d": sel2,
                "zeros64": np.zeros((DK, S), dtype=np.float32),
            }
        )
    return in_maps


LAST_EXEC_NS = None


def kernel(Q, K, V, W_Q, W_K, W_V, W_O, mask):
    global LAST_EXEC_NS
    causal = bool(np.asarray(mask).item())
    nc = _get_nc(causal)
    in_maps = make_in_maps(
        dict(Q=Q, K=K, V=V, W_Q=W_Q, W_K=W_K, W_V=W_V, W_O=W_O)
    )

    trace = bool(int(os.environ.get("ATTN_TRACE", "0")))
    res = run_bass_kernel_spmd(
        nc, in_maps, core_ids=list(range(NCORES)), trace=trace
    )
    LAST_EXEC_NS = res.exec_time_ns

    out = np.zeros((S, D), dtype=np.float32)
    for c in range(NCORES):
        out += np.asarray(res.results[c]["y0"], dtype=np.float32)
    return out
